# revision 4
# baseline (speedup 1.0000x reference)
"""Trainium2 Bass kernel: pointwise-conv (GEMM) + BatchNorm (folded) + LIF scan
+ spike-rate mean, sharded over 8 NeuronCores by TIME chunks.

Reference semantics (fp32):
    y   = einsum('bct,oc->bot', x, W)                   # [B, Cout, T]
    yb  = (y - mean) * (rsqrt(var+eps) * gamma) + beta  # BN (inference)
    v' = v + (yb_t - v)/2 ; s = (v' >= 1) ; v'' = v' * (1-s)   # LIF, T steps
    out = mean_t(s)                                     # [B, Cout]

Key facts exploited:
  * BN + the 1/TAU charge factor fold into the conv weights on the host:
        z_t = (0.5*gamma*rsqrt(var+eps) * W) @ x_t + bias
    and the LIF step becomes  u = 0.5*v + z ; spike = u>=1 ; v = u*(u<1).
  * The LIF recurrence forgets its state at rate 0.5/step (hard reset only
    accelerates forgetting), so a time chunk can be computed exactly from a
    zero state started WARM steps earlier: state influence is ~0.5^WARM and
    decays a further 0.5/step inside the owned window, so threshold flips
    require near-exact ties (measured: no additional absmax error at
    WARM=16).  Time-sharding is thus embarrassingly parallel with a
    WARM-step overlap.  (Core 0's warmup input columns are zeroed so its
    state stays exactly 0.)
  * fp32 matmuls run at 1/4 PE rate; instead split both operands into
    bf16 hi+lo and take 3 bf16 matmuls (hi*hi + hi*lo + lo*hi), all at
    full PE rate with fp32 PSUM accumulation.  Residual ~2^-18 relative —
    empirically the same single spike-flip vs the jax reference as an
    exact fp32 matmul (the flip comes from summation-order rounding).
    Also halves DMA bytes (bf16 pairs = 4B per original fp32 element).
  * scalar_tensor_tensor (DVE) computes (in0 op0 scalar) op1 in1 in ONE op,
    so each LIF step is 3 fused DVE ops at free-dim 128:
        u   = (v  * 0.5) + z        (mult, add)
        acc = (u >= 1.0) + acc      (is_ge, add)     [owned steps only]
        v   = (u <  1.0) * u        (is_lt, mult)    <- hard reset fused
    The scan overlaps the next window's matmul/DMA almost entirely.

Per core c (of 8): time window [128c - WARM, 128c + 128); spikes counted
only for the core's own 128 steps; host sums the 8 partial counts / 1024.

Measured on trn2 (For_i-loop delta timing): ~132 us/run end-to-end per
core; pipeline is co-limited by PE (3x bf16 matmul streams, ~95-110 us),
the DVE scan (~110 us busy, mostly hidden), and the ~19 MB x DMA (~57 us).
Window size 8, warmup 16, PSUM 4-deep (all 8 banks), x/z double/triple
buffered; deeper SBUF buffering measured slower (semaphore churn).
"""

import sys
import numpy as np

if "/opt/trn_rl_repo" not in sys.path:
    sys.path.insert(0, "/opt/trn_rl_repo")

# --- problem constants (hardcoded; kernel.py must be self-contained) ---
B, CIN, T, COUT = 64, 512, 1024, 256
NCORES = 8
WARM = 16                    # warmup steps per core (state influence 0.5^16
                             # ~ 1.5e-5; flips need |u-1| < ~1e-5 * 0.5^k at
                             # owned step k -- measured 0 extra absmax error)
TCH = T // NCORES            # 128 owned steps / core
TLOC = WARM + TCH            # 144 local steps
TS = 8                       # time-steps per matmul/psum window
NTS = TLOC // TS             # 18 windows
KO = CIN // 128              # 4 contraction chunks
BN_EPS = 1e-5

_CACHE = {}

# "bf16x3": 3 bf16 hi/lo-split matmuls at full PE rate (default)
# "f32"   : exact fp32 matmuls (4 cycles/row on PE)
MM_MODE = "bf16x3"
# engine that accumulates the spike count:
#   "vector"     - 3rd fused STT op per step on DVE
#   "pool_block" - per-window block on GPSIMD: is_ge mask + tree-add
#                  (keeps DVE at 2 ops/step; Pool is otherwise idle)
COUNT_ENGINE = "pool_block"
# scan emission: "full" = one STT per op over [128, 2, B] (free 128);
# "ilv2" = split the ch axis into 2 independent streams of free 64 and
# interleave their ops so same-engine semaphore latency (~100ns) hides
# behind the other stream's op -> no DVE stalls between dependent ops.
SCAN_MODE = "ilv2"


def _build_nc(with_bias: bool, mm_mode: str = MM_MODE,
              count_engine: str = COUNT_ENGINE, reps: int = 1,
              loop_reps: int = 0, warm: int = WARM,
              scan_ops: int = 3, mm_terms: int = 0,
              skip_mm: bool = False, skip_evac: bool = False,
              ts: int = TS, bufs: tuple = (2, 3, 4), taper: bool = False,
              fuse_dma: bool = True, evac_merge: bool = False,
              scan_mode: str = SCAN_MODE):
    import concourse.tile as tile
    from concourse import bacc, mybir

    f32 = mybir.dt.float32
    bf16 = mybir.dt.bfloat16
    op = mybir.AluOpType
    split = mm_mode == "bf16x3"
    x_dt = bf16 if split else f32
    nhl = 2 if split else 1

    nc = bacc.Bacc(None)
    # per-core inputs, host-prearranged so every DMA is one contiguous block:
    #   xk [KO, 128, NTS, nhl, B, TS]  (nhl=2: bf16 hi/lo split of x)
    #   wT [nhl, CIN, COUT]            (folded weights, k-major)
    if taper:
        # per-window contiguous blocks, concatenated along the free axis
        xk = nc.declare_dram_parameter(
            "xk", [128, KO * nhl * B * (warm + TCH)], x_dt, isOutput=False)
    elif fuse_dma:
        # all KO chunks of a window in one contiguous 8KB/partition block
        nts = (warm + TCH) // ts
        xk = nc.declare_dram_parameter("xk", [128, nts, KO, nhl, B, ts], x_dt,
                                       isOutput=False)
    else:
        nts = (warm + TCH) // ts
        xk = nc.declare_dram_parameter("xk", [KO, 128, nts, nhl, B, ts], x_dt,
                                       isOutput=False)
    wT = nc.declare_dram_parameter("wT", [nhl, CIN, COUT], x_dt, isOutput=False)
    if with_bias:
        bvec = nc.declare_dram_parameter("bvec", [1, 2, 128], f32, isOutput=False)
    counts = nc.declare_dram_parameter("counts", [128, 2, B], f32, isOutput=True)

    with tile.TileContext(nc) as tc:
        with (
            tc.tile_pool(name="consts", bufs=1) as consts,
            tc.tile_pool(name="xs", bufs=bufs[0]) as xs,
            tc.tile_pool(name="zs", bufs=bufs[1]) as zs,
            tc.tile_pool(name="psum", bufs=bufs[2], space="PSUM") as psum,
        ):
            # folded weights: [ki, hl, ko, m] with m = ch*128 + mi
            w_sb = consts.tile([128, nhl, KO, COUT], x_dt)
            nc.sync.dma_start(
                w_sb, wT.rearrange("h (ko ki) m -> ki h ko m", ki=128))

            bias_sb = ones_sb = None
            if with_bias:
                bias_sb = consts.tile([1, 2, 128], f32)
                nc.sync.dma_start(bias_sb, bvec[:])
                ones_sb = consts.tile([1, min(B, 512 // ts) * ts], f32)
                nc.vector.memset(ones_sb, 1.0)

            v = consts.tile([128, 2, B], f32)
            acc = consts.tile([128, 2, B], f32)

            # reps>1 / loop_reps>0 repeat the compute for benchmarking only
            if loop_reps > 0:
                with tc.For_i(0, loop_reps, 1):
                    _emit_body(nc, tc, xs, zs, psum, xk, counts, w_sb, v, acc,
                               bias_sb, ones_sb, split, count_engine, op, f32,
                               x_dt, mybir, warm, scan_ops, mm_terms,
                               skip_mm, skip_evac, ts, taper, fuse_dma,
                               evac_merge, scan_mode)
            else:
                for _rep in range(reps):
                    _emit_body(nc, tc, xs, zs, psum, xk, counts, w_sb, v, acc,
                               bias_sb, ones_sb, split, count_engine, op, f32,
                               x_dt, mybir, warm, scan_ops, mm_terms,
                               skip_mm, skip_evac, ts, taper, fuse_dma,
                               evac_merge, scan_mode)

    if not nc.is_finalized():
        nc.finalize()
    return nc


def _emit_body(nc, tc, xs, zs, psum, xk, counts, w_sb, v, acc,
               bias_sb, ones_sb, split, count_engine, op, f32, x_dt, mybir,
               warm=WARM, scan_ops=3, mm_terms=0,
               skip_mm=False, skip_evac=False, ts=TS, taper=False,
               fuse_dma=True, evac_merge=False, scan_mode=SCAN_MODE):
    with_bias = bias_sb is not None
    nhl = 2 if split else 1
    # (w_half, x_half) term list: hi*hi + hi*lo + lo*hi
    terms = [(0, 0), (0, 1), (1, 0)] if split else [(0, 0)]
    if mm_terms:
        terms = terms[:mm_terms]

    nc.vector.memset(v, 0.0)
    nc.vector.memset(acc, 0.0)

    tloc = warm + TCH
    if taper:
        # start-only taper: scan pipeline fills faster; PE extra cost sits
        # in the otherwise-idle head
        windows = [4, 4] + [8] * ((tloc - 8) // 8)
        assert sum(windows) == tloc
    else:
        windows = [ts] * (tloc // ts)
    t_base = 0
    for tsi, tsw in enumerate(windows):
        nbb = min(B, 512 // tsw)
        # ---- load x window (contig; one DMA covers all KO if fuse_dma) ----
        if taper:
            xta = xs.tile([128, KO, nhl, B, tsw], x_dt, tag=f"xa_{tsw}")
            off = KO * nhl * B * t_base
            sz = KO * nhl * B * tsw
            nc.sync.dma_start(
                xta, xk[:, off:off + sz].rearrange(
                    "p (ko h b t) -> p ko h b t", ko=KO, h=nhl, b=B))
            xts = [xta[:, ko] for ko in range(KO)]
        elif fuse_dma:
            xta = xs.tile([128, KO, nhl, B, tsw], x_dt, tag=f"xa_{tsw}")
            nc.sync.dma_start(xta, xk[:, tsi])
            xts = [xta[:, ko] for ko in range(KO)]
        else:
            xts = []
            for ko in range(KO):
                xt = xs.tile([128, nhl, B, tsw], x_dt, tag=f"x{ko}_{tsw}")
                nc.sync.dma_start(xt, xk[ko, :, tsi])
                xts.append(xt)

        # ---- matmul: psum[:, ch, (b,t)] += W'.T @ x  (split terms) ----
        pt = psum.tile([128, 2, B * tsw], f32, tag="pt")
        for ch in range(2) if not skip_mm else ():
            n_acc = len(terms) * KO
            i_acc = 0
            for ko in range(KO):
                for (wh, xh) in terms:
                    lhsT = w_sb[:, wh, ko, ch * 128:(ch + 1) * 128]
                    first = i_acc == 0
                    last = i_acc == n_acc - 1
                    i_acc += 1
                    for nb in range(B // nbb):
                        nc.tensor.matmul(
                            pt[:, ch, nb * nbb * tsw:(nb + 1) * nbb * tsw],
                            lhsT,
                            xts[ko][:, xh, nb * nbb:(nb + 1) * nbb, :],
                            start=first,
                            stop=(last and not with_bias),
                        )
            if with_bias:
                for nb in range(B // nbb):
                    nc.tensor.matmul(
                        pt[:, ch, nb * nbb * tsw:(nb + 1) * nbb * tsw],
                        bias_sb[:, ch, :],
                        ones_sb[:, :nbb * tsw],
                        start=False,
                        stop=True,
                    )

        # ---- evacuate psum -> sbuf z-block [128, TS, 2, B] (ACT) ----
        zb = zs.tile([128, tsw, 2, B], f32, tag=f"zb{tsw}")
        if not (skip_mm or skip_evac):
            if evac_merge:
                nc.scalar.copy(
                    out=zb[:],
                    in_=pt.rearrange("p c (b t) -> p t c b", t=tsw),
                )
            else:
                for ch in range(2):
                    nc.scalar.copy(
                        out=zb[:, :, ch, :],
                        in_=pt[:, ch].rearrange("p (b t) -> p t b", t=tsw),
                    )

        # ---- LIF scan: 2 fused DVE ops/step (+ count) ----
        for ti in range(tsw):
            t = t_base + ti
            u = zb[:, ti]  # holds z_t; overwritten in place with u_t
            if scan_mode == "ilv2":
                # two independent ch streams; interleave so each op's sem
                # latency hides behind the other stream's op
                if scan_ops >= 1:
                    for ch in range(2):
                        nc.vector.scalar_tensor_tensor(
                            out=u[:, ch], in0=v[:, ch], scalar=0.5,
                            in1=u[:, ch], op0=op.mult, op1=op.add,
                        )
                if scan_ops >= 3 and t >= warm and count_engine == "vector":
                    for ch in range(2):
                        nc.vector.scalar_tensor_tensor(
                            out=acc[:, ch], in0=u[:, ch], scalar=1.0,
                            in1=acc[:, ch], op0=op.is_ge, op1=op.add,
                        )
                if scan_ops >= 2:
                    for ch in range(2):
                        nc.vector.scalar_tensor_tensor(
                            out=v[:, ch], in0=u[:, ch], scalar=1.0,
                            in1=u[:, ch], op0=op.is_lt, op1=op.mult,
                        )
                continue
            if scan_ops >= 1:
                nc.vector.scalar_tensor_tensor(
                    out=u, in0=v, scalar=0.5, in1=u,
                    op0=op.mult, op1=op.add,
                )
            if scan_ops >= 3 and t >= warm and count_engine == "vector":
                nc.vector.scalar_tensor_tensor(
                    out=acc, in0=u, scalar=1.0, in1=acc,
                    op0=op.is_ge, op1=op.add,
                )
            if scan_ops >= 2:
                nc.vector.scalar_tensor_tensor(
                    out=v, in0=u, scalar=1.0, in1=u,
                    op0=op.is_lt, op1=op.mult,
                )

        if count_engine == "pool_block" and t_base >= warm:
            # zb still holds all 16 u_t tiles; count spikes on GPSIMD
            mblk = zs.tile([128, tsw, 2, B], f32, tag="mblk")
            nc.gpsimd.tensor_scalar(
                out=mblk[:], in0=zb[:], scalar1=1.0, scalar2=None,
                op0=op.is_ge,
            )
            h = tsw
            while h > 1:
                h //= 2
                nc.gpsimd.tensor_tensor(
                    out=mblk[:, :h], in0=mblk[:, :h], in1=mblk[:, h:2 * h],
                    op=op.add,
                )
            nc.gpsimd.tensor_tensor(
                out=acc, in0=acc, in1=mblk[:, 0], op=op.add,
            )

        t_base += tsw

    nc.sync.dma_start(counts[:], acc)


def _split_bf16(a):
    """fp32 -> (hi, lo) bf16 pair with hi + lo ~ a (error ~2^-18 relative)."""
    import ml_dtypes
    hi = a.astype(ml_dtypes.bfloat16)
    lo = (a - hi.astype(np.float32)).astype(ml_dtypes.bfloat16)
    return hi, lo


def _prep_inputs(x, W, gamma, beta, run_mean, run_var, mm_mode=None,
                 warm=WARM, ts=TS, taper=False, fuse_dma=True):
    """Fold BN + 1/TAU into weights; build per-core time-sharded x layouts."""
    if mm_mode is None:
        mm_mode = MM_MODE
    tloc = warm + TCH
    nts = tloc // 4 if taper else tloc // ts
    tsz = 4 if taper else ts
    split = mm_mode == "bf16x3"
    import ml_dtypes

    inv = 1.0 / np.sqrt(run_var.astype(np.float64) + BN_EPS)
    a = (0.5 * gamma.astype(np.float64) * inv)
    Wp = (W.astype(np.float64) * a[:, None]).astype(np.float32)       # [COUT, CIN]
    bp = (0.5 * (beta.astype(np.float64)
                 - run_mean.astype(np.float64) * gamma.astype(np.float64) * inv)
          ).astype(np.float32)                                        # [COUT]
    wT = np.ascontiguousarray(Wp.T)                                   # [CIN, COUT]
    if split:
        wh, wl = _split_bf16(wT)
        wTs = np.ascontiguousarray(np.stack([wh, wl], axis=0))        # [2,CIN,COUT]
        xh, xl = _split_bf16(x)
        xhl = np.stack([xh, xl], axis=0)                              # [2,B,CIN,T]
    else:
        wTs = wT.reshape(1, CIN, COUT)

    in_maps = []
    for c in range(NCORES):
        t0 = c * TCH - warm
        lo = max(t0, 0)
        if split:
            xc = np.zeros((2, B, CIN, tloc), dtype=ml_dtypes.bfloat16)
            xc[:, :, :, lo - t0:] = xhl[:, :, :, lo:c * TCH + TCH]
            # [2, B, CIN, tloc] -> [KO, 128, nts, 2, B, TS]
            if taper:
                # per-window contiguous blocks concatenated on the free axis
                blocks = []
                t0 = 0
                for tsw in [4, 4] + [8] * ((tloc - 8) // 8):
                    blk = (xc[:, :, :, t0:t0 + tsw]
                           .reshape(2, B, KO, 128, tsw)
                           .transpose(3, 2, 0, 1, 4)
                           .reshape(128, -1))
                    blocks.append(blk)
                    t0 += tsw
                xkc = np.ascontiguousarray(np.concatenate(blocks, axis=1))
            elif fuse_dma:
                # [128, nts, KO, nhl, B, tsz]
                xkc = np.ascontiguousarray(
                    xc.reshape(2, B, KO, 128, nts, tsz)
                      .transpose(3, 4, 2, 0, 1, 5))
            else:
                xkc = np.ascontiguousarray(
                    xc.reshape(2, B, KO, 128, nts, tsz)
                      .transpose(2, 3, 4, 0, 1, 5))
        else:
            xc = np.zeros((B, CIN, tloc), dtype=np.float32)
            xc[:, :, lo - t0:] = x[:, :, lo:c * TCH + TCH]
            xkc = np.ascontiguousarray(
                xc.reshape(B, KO, 128, nts, tsz).transpose(1, 2, 3, 0, 4)
            )[:, :, :, None]
        m = {"xk": xkc, "wT": wTs}
        if np.any(bp != 0):
            m["bvec"] = np.ascontiguousarray(bp.reshape(1, 2, 128))
        in_maps.append(m)
    return in_maps, bool(np.any(bp != 0))


def _postprocess(results):
    total = np.zeros((128, 2, B), dtype=np.float64)
    for r in results:
        total += r["counts"].astype(np.float64)
    # counts[ci, ch, b] -> out[b, ch*128+ci]
    out = total.transpose(2, 1, 0).reshape(B, COUT) / float(T)
    return out.astype(np.float32)


def kernel(x, W, gamma, beta, run_mean, run_var, _trace=False):
    from concourse.bass_utils import run_bass_kernel_spmd

    x = np.asarray(x, dtype=np.float32)
    W = np.asarray(W, dtype=np.float32)
    gamma = np.asarray(gamma, dtype=np.float32)
    beta = np.asarray(beta, dtype=np.float32)
    run_mean = np.asarray(run_mean, dtype=np.float32)
    run_var = np.asarray(run_var, dtype=np.float32)

    in_maps, with_bias = _prep_inputs(x, W, gamma, beta, run_mean, run_var)
    key = ("nc", with_bias, MM_MODE, COUNT_ENGINE, SCAN_MODE)
    if key not in _CACHE:
        _CACHE[key] = _build_nc(with_bias)
    nc = _CACHE[key]

    res = run_bass_kernel_spmd(
        nc, in_maps, core_ids=list(range(NCORES)), trace=_trace
    )
    out = _postprocess(res.results)
    if _trace:
        return out, res
    return out


if __name__ == "__main__":
    rng = np.random.default_rng(0)
    x = rng.standard_normal((B, CIN, T), dtype=np.float32)
    W = (rng.standard_normal((COUT, CIN), dtype=np.float32) / np.sqrt(CIN)).astype(np.float32)
    out = kernel(x, W, np.ones(COUT, np.float32), np.zeros(COUT, np.float32),
                 np.zeros(COUT, np.float32), np.ones(COUT, np.float32))
    print(out.shape, out.dtype, out[:2, :4])



# revision 12
# speedup vs baseline: 2.8463x; 2.8463x over previous
"""Trainium2 Bass kernel: pointwise-conv (GEMM) + BatchNorm (folded) + LIF scan
+ spike-rate mean, sharded over 8 NeuronCores by TIME chunks.

Reference semantics (fp32):
    y   = einsum('bct,oc->bot', x, W)                   # [B, Cout, T]
    yb  = (y - mean) * (rsqrt(var+eps) * gamma) + beta  # BN (inference)
    v' = v + (yb_t - v)/2 ; s = (v' >= 1) ; v'' = v' * (1-s)   # LIF, T steps
    out = mean_t(s)                                     # [B, Cout]

Key facts exploited:
  * BN + the 1/TAU charge factor fold into the conv weights on the host:
        z_t = (0.5*gamma*rsqrt(var+eps) * W) @ x_t + bias
    and the LIF step becomes  u = 0.5*v + z ; spike = u>=1 ; v = u*(u<1).
  * The LIF recurrence forgets its state at rate 0.5/step (hard reset only
    accelerates forgetting), so a time chunk can be computed exactly from a
    zero state started WARM steps earlier: state influence is ~0.5^WARM and
    decays a further 0.5/step inside the owned window, so threshold flips
    require near-exact ties (measured: no additional absmax error at
    WARM=16).  Time-sharding is thus embarrassingly parallel with a
    WARM-step overlap.  (Core 0's warmup input columns are zeroed so its
    state stays exactly 0.)
  * fp32 matmuls run at 1/4 PE rate; instead split both operands into
    bf16 hi+lo and take 3 bf16 matmuls (hi*hi + hi*lo + lo*hi), all at
    full PE rate with fp32 PSUM accumulation.  Residual ~2^-18 relative —
    empirically the same single spike-flip vs the jax reference as an
    exact fp32 matmul (the flip comes from summation-order rounding).
    Also halves DMA bytes (bf16 pairs = 4B per original fp32 element).
  * scalar_tensor_tensor (DVE) computes (in0 op0 scalar) op1 in1 in ONE op,
    so each LIF step is 3 fused DVE ops at free-dim 128:
        u   = (v  * 0.5) + z        (mult, add)
        acc = (u >= 1.0) + acc      (is_ge, add)     [owned steps only]
        v   = (u <  1.0) * u        (is_lt, mult)    <- hard reset fused
    The scan overlaps the next window's matmul/DMA almost entirely.

Per core c (of 8): time window [128c - WARM, 128c + 128); spikes counted
only for the core's own 128 steps; host sums the 8 partial counts / 1024.

Measured on trn2 (two-loop For_i delta timing, L65 vs L577):
~132 us/iter.  HW decomposition (delta bench of stripped variants):
  x DMA alone ~59 us (hidden); PE matmul stream ~115 us busy (the
  512-row matmuls pay a serialized 128-cycle ldweights each: 640cyc/
  matmul, not 512); DVE scan chain ~113 us (782ns/step: 3x194ns STT +
  2x100ns same-engine semaphore-visibility gaps); For_i loop overhead
  ~0.  Two scheduling fixes got 155->132 us:
  * evac_merge="one": ONE merged ACT psum->sbuf copy per window instead
    of per-channel/per-step copies.  ACT per-op overhead on HW is ~1us
    (far above the cost model), so fewer/bigger ACT ops win (-19 us);
    finer-grained evac (per 1/2/4 steps) measured WORSE.
  * scan_mode="uva": emit the count op AFTER the reset op so it fills
    the ~100ns sem gap before the next charge instead of sitting on the
    u->v critical chain (-5 us).
Measured dead ends: GPSIMD/Pool ops in the loop cost ~4us EACH on HW
(pool_block count: +310 us); fp32r matmul is tf32-precision (err std
7e-5: ~440 spike flips, fails the <=1-flip budget); fp8 correction
terms cap at 2^-13 for the same reason; ts=16 windows, deeper SBUF
bufs, per-step evac, ilv2 ch-split scan, psum-direct scan all measured
0-20 us worse.  Window size 8, warmup 16, PSUM 4-deep, x/z double/
triple buffered.
"""

import sys
import numpy as np

if "/opt/trn_rl_repo" not in sys.path:
    sys.path.insert(0, "/opt/trn_rl_repo")

# --- problem constants (hardcoded; kernel.py must be self-contained) ---
B, CIN, T, COUT = 64, 512, 1024, 256
NCORES = 8
WARM = 16                    # warmup steps per core (state influence 0.5^16
                             # ~ 1.5e-5; flips need |u-1| < ~1e-5 * 0.5^k at
                             # owned step k -- measured 0 extra absmax error)
TCH = T // NCORES            # 128 owned steps / core
TLOC = WARM + TCH            # 144 local steps
TS = 8                       # time-steps per matmul/psum window
NTS = TLOC // TS             # 18 windows
KO = CIN // 128              # 4 contraction chunks
BN_EPS = 1e-5

_CACHE = {}

EVAC_MODE = "one"  # single merged ACT evac op per window (ACT per-op overhead dominates)
SCAN_SRC = "sbuf"  # "psum": charge op reads z straight from PSUM (no evac)

# "bf16x3": 3 bf16 hi/lo-split matmuls at full PE rate (default)
# "f32"   : exact fp32 matmuls (4 cycles/row on PE)
MM_MODE = "bf16x3"
# engine that accumulates the spike count:
#   "vector"     - 3rd fused STT op per step on DVE
#   "pool_block" - per-window block on GPSIMD: is_ge mask + tree-add
#                  (keeps DVE at 2 ops/step; Pool is otherwise idle)
COUNT_ENGINE = "vector"
# scan emission: "full" = one STT per op over [128, 2, B] (free 128);
# "ilv2" = split the ch axis into 2 independent streams of free 64 and
# interleave their ops so same-engine semaphore latency (~100ns) hides
# behind the other stream's op -> no DVE stalls between dependent ops.
SCAN_MODE = "uva"


def _build_nc(with_bias: bool, mm_mode: str = MM_MODE,
              count_engine: str = COUNT_ENGINE, reps: int = 1,
              loop_reps: int = 0, warm: int = WARM,
              scan_ops: int = 3, mm_terms: int = 0,
              skip_mm: bool = False, skip_evac: bool = False,
              ts: int = TS, bufs: tuple = (2, 3, 4), taper: bool = False,
              fuse_dma: bool = True, evac_merge=None,
              scan_mode: str = SCAN_MODE, skip_xdma: bool = False,
              staggered: bool = False, scan_src: str = "sbuf"):
    import concourse.tile as tile
    from concourse import bacc, mybir

    f32 = mybir.dt.float32
    bf16 = mybir.dt.bfloat16
    op = mybir.AluOpType
    split = mm_mode == "bf16x3"
    x_dt = bf16 if split else f32
    nhl = 2 if split else 1

    nc = bacc.Bacc(None)
    # per-core inputs, host-prearranged so every DMA is one contiguous block:
    #   xk [KO, 128, NTS, nhl, B, TS]  (nhl=2: bf16 hi/lo split of x)
    #   wT [nhl, CIN, COUT]            (folded weights, k-major)
    if taper:
        # per-window contiguous blocks, concatenated along the free axis
        xk = nc.declare_dram_parameter(
            "xk", [128, KO * nhl * B * (warm + TCH)], x_dt, isOutput=False)
    elif fuse_dma:
        # all KO chunks of a window in one contiguous 8KB/partition block
        nts = (warm + TCH) // ts
        xk = nc.declare_dram_parameter("xk", [128, nts, KO, nhl, B, ts], x_dt,
                                       isOutput=False)
    else:
        nts = (warm + TCH) // ts
        xk = nc.declare_dram_parameter("xk", [KO, 128, nts, nhl, B, ts], x_dt,
                                       isOutput=False)
    wT = nc.declare_dram_parameter("wT", [nhl, CIN, COUT], x_dt, isOutput=False)
    if with_bias:
        bvec = nc.declare_dram_parameter("bvec", [1, 2, 128], f32, isOutput=False)
    counts = nc.declare_dram_parameter("counts", [128, 2, B], f32, isOutput=True)

    with tile.TileContext(nc) as tc:
        with (
            tc.tile_pool(name="consts", bufs=1) as consts,
            tc.tile_pool(name="xs", bufs=bufs[0]) as xs,
            tc.tile_pool(name="zs", bufs=bufs[1]) as zs,
            tc.tile_pool(name="psum", bufs=bufs[2], space="PSUM") as psum,
        ):
            # folded weights: [ki, hl, ko, m] with m = ch*128 + mi
            w_sb = consts.tile([128, nhl, KO, COUT], x_dt)
            nc.sync.dma_start(
                w_sb, wT.rearrange("h (ko ki) m -> ki h ko m", ki=128))

            bias_sb = ones_sb = None
            if with_bias:
                bias_sb = consts.tile([1, 2, 128], f32)
                nc.sync.dma_start(bias_sb, bvec[:])
                ones_sb = consts.tile([1, min(B, 512 // ts) * ts], f32)
                nc.vector.memset(ones_sb, 1.0)

            v = consts.tile([128, 2, B], f32)
            acc = consts.tile([128, 2, B], f32)

            # reps>1 / loop_reps>0 repeat the compute for benchmarking only
            if loop_reps > 0:
                with tc.For_i(0, loop_reps, 1, staggered_reset=staggered):
                    _emit_body(nc, tc, xs, zs, psum, xk, counts, w_sb, v, acc,
                               bias_sb, ones_sb, split, count_engine, op, f32,
                               x_dt, mybir, warm, scan_ops, mm_terms,
                               skip_mm, skip_evac, ts, taper, fuse_dma,
                               evac_merge, scan_mode, skip_xdma, scan_src)
            else:
                for _rep in range(reps):
                    _emit_body(nc, tc, xs, zs, psum, xk, counts, w_sb, v, acc,
                               bias_sb, ones_sb, split, count_engine, op, f32,
                               x_dt, mybir, warm, scan_ops, mm_terms,
                               skip_mm, skip_evac, ts, taper, fuse_dma,
                               evac_merge, scan_mode, skip_xdma, scan_src)

    if not nc.is_finalized():
        nc.finalize()
    return nc


def _emit_body(nc, tc, xs, zs, psum, xk, counts, w_sb, v, acc,
               bias_sb, ones_sb, split, count_engine, op, f32, x_dt, mybir,
               warm=WARM, scan_ops=3, mm_terms=0,
               skip_mm=False, skip_evac=False, ts=TS, taper=False,
               fuse_dma=True, evac_merge=None, scan_mode=SCAN_MODE,
               skip_xdma=False, scan_src="sbuf"):
    with_bias = bias_sb is not None
    nhl = 2 if split else 1
    # (w_half, x_half) term list: hi*hi + hi*lo + lo*hi
    terms = [(0, 0), (0, 1), (1, 0)] if split else [(0, 0)]
    if mm_terms:
        terms = terms[:mm_terms]

    nc.vector.memset(v, 0.0)
    nc.vector.memset(acc, 0.0)

    tloc = warm + TCH
    if taper:
        # start-only taper: scan pipeline fills faster; PE extra cost sits
        # in the otherwise-idle head
        windows = [4, 4] + [8] * ((tloc - 8) // 8)
        assert sum(windows) == tloc
    else:
        windows = [ts] * (tloc // ts)
    t_base = 0
    for tsi, tsw in enumerate(windows):
        nbb = min(B, 512 // tsw)
        # ---- load x window (contig; one DMA covers all KO if fuse_dma) ----
        if taper:
            xta = xs.tile([128, KO, nhl, B, tsw], x_dt, tag=f"xa_{tsw}")
            off = KO * nhl * B * t_base
            sz = KO * nhl * B * tsw
            nc.sync.dma_start(
                xta, xk[:, off:off + sz].rearrange(
                    "p (ko h b t) -> p ko h b t", ko=KO, h=nhl, b=B))
            xts = [xta[:, ko] for ko in range(KO)]
        elif fuse_dma:
            xta = xs.tile([128, KO, nhl, B, tsw], x_dt, tag=f"xa_{tsw}")
            if not skip_xdma:
                nc.sync.dma_start(xta, xk[:, tsi])
            else:
                nc.vector.memset(xta[:, 0, 0, 0], 0.0)
            xts = [xta[:, ko] for ko in range(KO)]
        else:
            xts = []
            for ko in range(KO):
                xt = xs.tile([128, nhl, B, tsw], x_dt, tag=f"x{ko}_{tsw}")
                nc.sync.dma_start(xt, xk[ko, :, tsi])
                xts.append(xt)

        # ---- matmul: psum[:, ch, (b,t)] += W'.T @ x  (split terms) ----
        pt = psum.tile([128, 2, B * tsw], f32, tag="pt")
        for ch in range(2) if not skip_mm else ():
            n_acc = len(terms) * KO
            i_acc = 0
            for ko in range(KO):
                for (wh, xh) in terms:
                    lhsT = w_sb[:, wh, ko, ch * 128:(ch + 1) * 128]
                    first = i_acc == 0
                    last = i_acc == n_acc - 1
                    i_acc += 1
                    for nb in range(B // nbb):
                        nc.tensor.matmul(
                            pt[:, ch, nb * nbb * tsw:(nb + 1) * nbb * tsw],
                            lhsT,
                            xts[ko][:, xh, nb * nbb:(nb + 1) * nbb, :],
                            start=first,
                            stop=(last and not with_bias),
                        )
            if with_bias:
                for nb in range(B // nbb):
                    nc.tensor.matmul(
                        pt[:, ch, nb * nbb * tsw:(nb + 1) * nbb * tsw],
                        bias_sb[:, ch, :],
                        ones_sb[:, :nbb * tsw],
                        start=False,
                        stop=True,
                    )

        # ---- evacuate psum -> sbuf z-block [128, TS, 2, B] (ACT) ----
        # (scan_src == "psum": no evac; the scan's charge op reads z
        #  straight out of PSUM and writes u into zb)
        zb = zs.tile([128, tsw, 2, B], f32, tag=f"zb{tsw}")
        ptv = pt.rearrange("p c (b t) -> p t c b", t=tsw)
        if not (skip_mm or skip_evac) and scan_src != "psum":
            if evac_merge is None:
                evac_merge = "ch"
            if evac_merge == "one":
                nc.scalar.copy(
                    out=zb[:],
                    in_=pt.rearrange("p c (b t) -> p t c b", t=tsw),
                )
            elif isinstance(evac_merge, str) and evac_merge.startswith("t"):
                # per-chunk-of-steps evac: scan step ti only waits its own
                # chunk, so ACT latency pipelines off the DVE chain
                tn = int(evac_merge[1:])
                ptv = pt.rearrange("p c (b t) -> p t c b", t=tsw)
                for t0 in range(0, tsw, tn):
                    nc.scalar.copy(
                        out=zb[:, t0:t0 + tn],
                        in_=ptv[:, t0:t0 + tn],
                    )
            else:
                for ch in range(2):
                    nc.scalar.copy(
                        out=zb[:, :, ch, :],
                        in_=pt[:, ch].rearrange("p (b t) -> p t b", t=tsw),
                    )

        # ---- LIF scan: 2 fused DVE ops/step (+ count) ----
        for ti in range(tsw):
            t = t_base + ti
            u = zb[:, ti]  # holds z_t; overwritten in place with u_t
            zsrc = ptv[:, ti] if scan_src == "psum" else u
            if scan_mode == "ilv2":
                # two independent ch streams; interleave so each op's sem
                # latency hides behind the other stream's op
                if scan_ops >= 1:
                    for ch in range(2):
                        nc.vector.scalar_tensor_tensor(
                            out=u[:, ch], in0=v[:, ch], scalar=0.5,
                            in1=u[:, ch], op0=op.mult, op1=op.add,
                        )
                if scan_ops >= 3 and t >= warm and count_engine == "vector":
                    for ch in range(2):
                        nc.vector.scalar_tensor_tensor(
                            out=acc[:, ch], in0=u[:, ch], scalar=1.0,
                            in1=acc[:, ch], op0=op.is_ge, op1=op.add,
                        )
                if scan_ops >= 2:
                    for ch in range(2):
                        nc.vector.scalar_tensor_tensor(
                            out=v[:, ch], in0=u[:, ch], scalar=1.0,
                            in1=u[:, ch], op0=op.is_lt, op1=op.mult,
                        )
                continue
            do_count = scan_ops >= 3 and t >= warm and count_engine == "vector"
            if scan_ops >= 1:
                nc.vector.scalar_tensor_tensor(
                    out=u, in0=v, scalar=0.5, in1=zsrc,
                    op0=op.mult, op1=op.add,
                )
            if do_count and scan_mode != "uva":
                nc.vector.scalar_tensor_tensor(
                    out=acc, in0=u, scalar=1.0, in1=acc,
                    op0=op.is_ge, op1=op.add,
                )
            if scan_ops >= 2:
                nc.vector.scalar_tensor_tensor(
                    out=v, in0=u, scalar=1.0, in1=u,
                    op0=op.is_lt, op1=op.mult,
                )
            if do_count and scan_mode == "uva":
                # count emitted AFTER the reset: it fills the ~100ns sem
                # gap before the next step's charge op instead of sitting
                # on the u->v critical chain
                nc.vector.scalar_tensor_tensor(
                    out=acc, in0=u, scalar=1.0, in1=acc,
                    op0=op.is_ge, op1=op.add,
                )

        if count_engine == "pool_block" and t_base >= warm:
            # zb still holds all 16 u_t tiles; count spikes on GPSIMD
            mblk = zs.tile([128, tsw, 2, B], f32, tag="mblk")
            nc.gpsimd.tensor_scalar(
                out=mblk[:], in0=zb[:], scalar1=1.0, scalar2=None,
                op0=op.is_ge,
            )
            h = tsw
            while h > 1:
                h //= 2
                nc.gpsimd.tensor_tensor(
                    out=mblk[:, :h], in0=mblk[:, :h], in1=mblk[:, h:2 * h],
                    op=op.add,
                )
            nc.gpsimd.tensor_tensor(
                out=acc, in0=acc, in1=mblk[:, 0], op=op.add,
            )

        t_base += tsw

    nc.sync.dma_start(counts[:], acc)


def _split_bf16(a):
    """fp32 -> (hi, lo) bf16 pair with hi + lo ~ a (error ~2^-18 relative)."""
    import ml_dtypes
    hi = a.astype(ml_dtypes.bfloat16)
    lo = (a - hi.astype(np.float32)).astype(ml_dtypes.bfloat16)
    return hi, lo


def _prep_inputs(x, W, gamma, beta, run_mean, run_var, mm_mode=None,
                 warm=WARM, ts=TS, taper=False, fuse_dma=True):
    """Fold BN + 1/TAU into weights; build per-core time-sharded x layouts."""
    if mm_mode is None:
        mm_mode = MM_MODE
    tloc = warm + TCH
    nts = tloc // 4 if taper else tloc // ts
    tsz = 4 if taper else ts
    split = mm_mode == "bf16x3"
    import ml_dtypes

    inv = 1.0 / np.sqrt(run_var.astype(np.float64) + BN_EPS)
    a = (0.5 * gamma.astype(np.float64) * inv)
    Wp = (W.astype(np.float64) * a[:, None]).astype(np.float32)       # [COUT, CIN]
    bp = (0.5 * (beta.astype(np.float64)
                 - run_mean.astype(np.float64) * gamma.astype(np.float64) * inv)
          ).astype(np.float32)                                        # [COUT]
    wT = np.ascontiguousarray(Wp.T)                                   # [CIN, COUT]
    if split:
        wh, wl = _split_bf16(wT)
        wTs = np.ascontiguousarray(np.stack([wh, wl], axis=0))        # [2,CIN,COUT]
        xh, xl = _split_bf16(x)
        xhl = np.stack([xh, xl], axis=0)                              # [2,B,CIN,T]
    else:
        wTs = wT.reshape(1, CIN, COUT)

    in_maps = []
    for c in range(NCORES):
        t0 = c * TCH - warm
        lo = max(t0, 0)
        if split:
            xc = np.zeros((2, B, CIN, tloc), dtype=ml_dtypes.bfloat16)
            xc[:, :, :, lo - t0:] = xhl[:, :, :, lo:c * TCH + TCH]
            # [2, B, CIN, tloc] -> [KO, 128, nts, 2, B, TS]
            if taper:
                # per-window contiguous blocks concatenated on the free axis
                blocks = []
                t0 = 0
                for tsw in [4, 4] + [8] * ((tloc - 8) // 8):
                    blk = (xc[:, :, :, t0:t0 + tsw]
                           .reshape(2, B, KO, 128, tsw)
                           .transpose(3, 2, 0, 1, 4)
                           .reshape(128, -1))
                    blocks.append(blk)
                    t0 += tsw
                xkc = np.ascontiguousarray(np.concatenate(blocks, axis=1))
            elif fuse_dma:
                # [128, nts, KO, nhl, B, tsz]
                xkc = np.ascontiguousarray(
                    xc.reshape(2, B, KO, 128, nts, tsz)
                      .transpose(3, 4, 2, 0, 1, 5))
            else:
                xkc = np.ascontiguousarray(
                    xc.reshape(2, B, KO, 128, nts, tsz)
                      .transpose(2, 3, 4, 0, 1, 5))
        else:
            xc = np.zeros((B, CIN, tloc), dtype=np.float32)
            xc[:, :, lo - t0:] = x[:, :, lo:c * TCH + TCH]
            xkc = np.ascontiguousarray(
                xc.reshape(B, KO, 128, nts, tsz).transpose(1, 2, 3, 0, 4)
            )[:, :, :, None]
        m = {"xk": xkc, "wT": wTs}
        if np.any(bp != 0):
            m["bvec"] = np.ascontiguousarray(bp.reshape(1, 2, 128))
        in_maps.append(m)
    return in_maps, bool(np.any(bp != 0))


def _postprocess(results):
    total = np.zeros((128, 2, B), dtype=np.float64)
    for r in results:
        total += r["counts"].astype(np.float64)
    # counts[ci, ch, b] -> out[b, ch*128+ci]
    out = total.transpose(2, 1, 0).reshape(B, COUT) / float(T)
    return out.astype(np.float32)


def kernel(x, W, gamma, beta, run_mean, run_var, _trace=False):
    from concourse.bass_utils import run_bass_kernel_spmd

    x = np.asarray(x, dtype=np.float32)
    W = np.asarray(W, dtype=np.float32)
    gamma = np.asarray(gamma, dtype=np.float32)
    beta = np.asarray(beta, dtype=np.float32)
    run_mean = np.asarray(run_mean, dtype=np.float32)
    run_var = np.asarray(run_var, dtype=np.float32)

    in_maps, with_bias = _prep_inputs(x, W, gamma, beta, run_mean, run_var)
    key = ("nc", with_bias, MM_MODE, COUNT_ENGINE, SCAN_MODE, EVAC_MODE,
           SCAN_SRC)
    if key not in _CACHE:
        _CACHE[key] = _build_nc(with_bias, evac_merge=EVAC_MODE,
                                scan_src=SCAN_SRC)
    nc = _CACHE[key]

    res = run_bass_kernel_spmd(
        nc, in_maps, core_ids=list(range(NCORES)), trace=_trace
    )
    out = _postprocess(res.results)
    if _trace:
        return out, res
    return out


if __name__ == "__main__":
    rng = np.random.default_rng(0)
    x = rng.standard_normal((B, CIN, T), dtype=np.float32)
    W = (rng.standard_normal((COUT, CIN), dtype=np.float32) / np.sqrt(CIN)).astype(np.float32)
    out = kernel(x, W, np.ones(COUT, np.float32), np.zeros(COUT, np.float32),
                 np.zeros(COUT, np.float32), np.ones(COUT, np.float32))
    print(out.shape, out.dtype, out[:2, :4])



# revision 15
# speedup vs baseline: 2.8564x; 1.0035x over previous
"""Trainium2 Bass kernel: pointwise-conv (GEMM) + BatchNorm (folded) + LIF scan
+ spike-rate mean, sharded over 8 NeuronCores by TIME chunks.

Reference semantics (fp32):
    y   = einsum('bct,oc->bot', x, W)                   # [B, Cout, T]
    yb  = (y - mean) * (rsqrt(var+eps) * gamma) + beta  # BN (inference)
    v' = v + (yb_t - v)/2 ; s = (v' >= 1) ; v'' = v' * (1-s)   # LIF, T steps
    out = mean_t(s)                                     # [B, Cout]

Key facts exploited:
  * BN + the 1/TAU charge factor fold into the conv weights on the host:
        z_t = (0.5*gamma*rsqrt(var+eps) * W) @ x_t + bias
    and the LIF step becomes  u = 0.5*v + z ; spike = u>=1 ; v = u*(u<1).
  * The LIF recurrence forgets its state at rate 0.5/step (hard reset only
    accelerates forgetting), so a time chunk can be computed exactly from a
    zero state started WARM steps earlier: state influence is ~0.5^WARM and
    decays a further 0.5/step inside the owned window, so threshold flips
    require near-exact ties (measured: no additional absmax error at
    WARM=16).  Time-sharding is thus embarrassingly parallel with a
    WARM-step overlap.  (Core 0's warmup input columns are zeroed so its
    state stays exactly 0.)
  * fp32 matmuls run at 1/4 PE rate; instead split both operands into
    bf16 hi+lo and take 3 bf16 matmuls (hi*hi + hi*lo + lo*hi), all at
    full PE rate with fp32 PSUM accumulation.  Residual ~2^-18 relative —
    empirically the same single spike-flip vs the jax reference as an
    exact fp32 matmul (the flip comes from summation-order rounding).
    Also halves DMA bytes (bf16 pairs = 4B per original fp32 element).
  * scalar_tensor_tensor (DVE) computes (in0 op0 scalar) op1 in1 in ONE op,
    so each LIF step is 3 fused DVE ops at free-dim 128:
        u   = (v  * 0.5) + z        (mult, add)
        acc = (u >= 1.0) + acc      (is_ge, add)     [owned steps only]
        v   = (u <  1.0) * u        (is_lt, mult)    <- hard reset fused
    The scan overlaps the next window's matmul/DMA almost entirely.

Per core c (of 8): time window [128c - WARM, 128c + 128); spikes counted
only for the core's own 128 steps; host sums the 8 partial counts / 1024.

Measured on trn2 (two-loop For_i delta timing, L65 vs L577):
~132 us/iter.  HW decomposition (delta bench of stripped variants):
  x DMA alone ~59 us (hidden); PE matmul stream ~115 us busy (the
  512-row matmuls pay a serialized 128-cycle ldweights each: 640cyc/
  matmul, not 512); DVE scan chain ~113 us (782ns/step: 3x194ns STT +
  2x100ns same-engine semaphore-visibility gaps); For_i loop overhead
  ~0.  Two scheduling fixes got 155->132 us:
  * evac_merge="one": ONE merged ACT psum->sbuf copy per window instead
    of per-channel/per-step copies.  ACT per-op overhead on HW is ~1us
    (far above the cost model), so fewer/bigger ACT ops win (-19 us);
    finer-grained evac (per 1/2/4 steps) measured WORSE.
  * scan_mode="uva": emit the count op AFTER the reset op so it fills
    the ~100ns sem gap before the next charge instead of sitting on the
    u->v critical chain (-5 us).
With 2 kernel bodies per For_i iteration the per-body time drops to
~118 us = the PE-stream floor: the remaining ~14 us at 1 body/iter is
the loop's semaphore-reset drain (engines must finish the scan tail
before sems reset), i.e. steady-state throughput is PE-bound at the
ldweights-inclusive floor.
Measured dead ends: GPSIMD/Pool ops in the loop cost ~4us EACH on HW
(pool_block count: +310 us); fp32r matmul is tf32-precision (err std
7e-5: ~440 spike flips, fails the <=1-flip budget); fp8 correction
terms cap at 2^-13 for the same reason; ts=16 windows, xs=3 bufs,
per-step evac, ilv2 ch-split scan, psum-direct scan all measured
0-20 us worse.  Window size 8, warmup 16, PSUM 4-deep, x double / z
quad buffered.
"""

import sys
import numpy as np

if "/opt/trn_rl_repo" not in sys.path:
    sys.path.insert(0, "/opt/trn_rl_repo")

# --- problem constants (hardcoded; kernel.py must be self-contained) ---
B, CIN, T, COUT = 64, 512, 1024, 256
NCORES = 8
WARM = 16                    # warmup steps per core (state influence 0.5^16
                             # ~ 1.5e-5; flips need |u-1| < ~1e-5 * 0.5^k at
                             # owned step k -- measured 0 extra absmax error)
TCH = T // NCORES            # 128 owned steps / core
TLOC = WARM + TCH            # 144 local steps
TS = 8                       # time-steps per matmul/psum window
NTS = TLOC // TS             # 18 windows
KO = CIN // 128              # 4 contraction chunks
BN_EPS = 1e-5

_CACHE = {}

EVAC_MODE = "one"  # single merged ACT evac op per window (ACT per-op overhead dominates)
SCAN_SRC = "sbuf"  # "psum": charge op reads z straight from PSUM (no evac)

# "bf16x3": 3 bf16 hi/lo-split matmuls at full PE rate (default)
# "f32"   : exact fp32 matmuls (4 cycles/row on PE)
MM_MODE = "bf16x3"
# engine that accumulates the spike count:
#   "vector"     - 3rd fused STT op per step on DVE
#   "pool_block" - per-window block on GPSIMD: is_ge mask + tree-add
#                  (keeps DVE at 2 ops/step; Pool is otherwise idle)
COUNT_ENGINE = "vector"
# scan emission: "full" = one STT per op over [128, 2, B] (free 128);
# "ilv2" = split the ch axis into 2 independent streams of free 64 and
# interleave their ops so same-engine semaphore latency (~100ns) hides
# behind the other stream's op -> no DVE stalls between dependent ops.
SCAN_MODE = "uva"


def _build_nc(with_bias: bool, mm_mode: str = MM_MODE,
              count_engine: str = COUNT_ENGINE, reps: int = 1,
              loop_reps: int = 0, warm: int = WARM,
              scan_ops: int = 3, mm_terms: int = 0,
              skip_mm: bool = False, skip_evac: bool = False,
              ts: int = TS, bufs: tuple = (2, 4, 4), taper: bool = False,
              fuse_dma: bool = True, evac_merge=None,
              scan_mode: str = SCAN_MODE, skip_xdma: bool = False,
              staggered: bool = False, scan_src: str = "sbuf",
              body_reps: int = 1):
    import concourse.tile as tile
    from concourse import bacc, mybir

    f32 = mybir.dt.float32
    bf16 = mybir.dt.bfloat16
    op = mybir.AluOpType
    split = mm_mode == "bf16x3"
    x_dt = bf16 if split else f32
    nhl = 2 if split else 1

    nc = bacc.Bacc(None)
    # per-core inputs, host-prearranged so every DMA is one contiguous block:
    #   xk [KO, 128, NTS, nhl, B, TS]  (nhl=2: bf16 hi/lo split of x)
    #   wT [nhl, CIN, COUT]            (folded weights, k-major)
    if taper:
        # per-window contiguous blocks, concatenated along the free axis
        xk = nc.declare_dram_parameter(
            "xk", [128, KO * nhl * B * (warm + TCH)], x_dt, isOutput=False)
    elif fuse_dma:
        # all KO chunks of a window in one contiguous 8KB/partition block
        nts = (warm + TCH) // ts
        xk = nc.declare_dram_parameter("xk", [128, nts, KO, nhl, B, ts], x_dt,
                                       isOutput=False)
    else:
        nts = (warm + TCH) // ts
        xk = nc.declare_dram_parameter("xk", [KO, 128, nts, nhl, B, ts], x_dt,
                                       isOutput=False)
    wT = nc.declare_dram_parameter("wT", [nhl, CIN, COUT], x_dt, isOutput=False)
    if with_bias:
        bvec = nc.declare_dram_parameter("bvec", [1, 2, 128], f32, isOutput=False)
    counts = nc.declare_dram_parameter("counts", [128, 2, B], f32, isOutput=True)

    with tile.TileContext(nc) as tc:
        with (
            tc.tile_pool(name="consts", bufs=1) as consts,
            tc.tile_pool(name="xs", bufs=bufs[0]) as xs,
            tc.tile_pool(name="zs", bufs=bufs[1]) as zs,
            tc.tile_pool(name="psum", bufs=bufs[2], space="PSUM") as psum,
        ):
            # folded weights: [ki, hl, ko, m] with m = ch*128 + mi
            w_sb = consts.tile([128, nhl, KO, COUT], x_dt)
            nc.sync.dma_start(
                w_sb, wT.rearrange("h (ko ki) m -> ki h ko m", ki=128))

            bias_sb = ones_sb = None
            if with_bias:
                bias_sb = consts.tile([1, 2, 128], f32)
                nc.sync.dma_start(bias_sb, bvec[:])
                ones_sb = consts.tile([1, min(B, 512 // ts) * ts], f32)
                nc.vector.memset(ones_sb, 1.0)

            v = consts.tile([128, 2, B], f32)
            acc = consts.tile([128, 2, B], f32)

            # reps>1 / loop_reps>0 repeat the compute for benchmarking only
            if loop_reps > 0:
                with tc.For_i(0, loop_reps, 1, staggered_reset=staggered):
                    for _br in range(body_reps):
                        _emit_body(nc, tc, xs, zs, psum, xk, counts, w_sb,
                                   v, acc, bias_sb, ones_sb, split,
                                   count_engine, op, f32, x_dt, mybir, warm,
                                   scan_ops, mm_terms, skip_mm, skip_evac,
                                   ts, taper, fuse_dma, evac_merge,
                                   scan_mode, skip_xdma, scan_src)
            else:
                for _rep in range(reps):
                    _emit_body(nc, tc, xs, zs, psum, xk, counts, w_sb, v, acc,
                               bias_sb, ones_sb, split, count_engine, op, f32,
                               x_dt, mybir, warm, scan_ops, mm_terms,
                               skip_mm, skip_evac, ts, taper, fuse_dma,
                               evac_merge, scan_mode, skip_xdma, scan_src)

    if not nc.is_finalized():
        nc.finalize()
    return nc


def _emit_body(nc, tc, xs, zs, psum, xk, counts, w_sb, v, acc,
               bias_sb, ones_sb, split, count_engine, op, f32, x_dt, mybir,
               warm=WARM, scan_ops=3, mm_terms=0,
               skip_mm=False, skip_evac=False, ts=TS, taper=False,
               fuse_dma=True, evac_merge=None, scan_mode=SCAN_MODE,
               skip_xdma=False, scan_src="sbuf"):
    with_bias = bias_sb is not None
    nhl = 2 if split else 1
    # (w_half, x_half) term list: hi*hi + hi*lo + lo*hi
    terms = [(0, 0), (0, 1), (1, 0)] if split else [(0, 0)]
    if mm_terms:
        terms = terms[:mm_terms]

    nc.vector.memset(v, 0.0)
    nc.vector.memset(acc, 0.0)

    tloc = warm + TCH
    if taper:
        # start-only taper: scan pipeline fills faster; PE extra cost sits
        # in the otherwise-idle head
        windows = [4, 4] + [8] * ((tloc - 8) // 8)
        assert sum(windows) == tloc
    else:
        windows = [ts] * (tloc // ts)
    t_base = 0
    for tsi, tsw in enumerate(windows):
        nbb = min(B, 512 // tsw)
        # ---- load x window (contig; one DMA covers all KO if fuse_dma) ----
        if taper:
            xta = xs.tile([128, KO, nhl, B, tsw], x_dt, tag=f"xa_{tsw}")
            off = KO * nhl * B * t_base
            sz = KO * nhl * B * tsw
            nc.sync.dma_start(
                xta, xk[:, off:off + sz].rearrange(
                    "p (ko h b t) -> p ko h b t", ko=KO, h=nhl, b=B))
            xts = [xta[:, ko] for ko in range(KO)]
        elif fuse_dma:
            xta = xs.tile([128, KO, nhl, B, tsw], x_dt, tag=f"xa_{tsw}")
            if not skip_xdma:
                nc.sync.dma_start(xta, xk[:, tsi])
            else:
                nc.vector.memset(xta[:, 0, 0, 0], 0.0)
            xts = [xta[:, ko] for ko in range(KO)]
        else:
            xts = []
            for ko in range(KO):
                xt = xs.tile([128, nhl, B, tsw], x_dt, tag=f"x{ko}_{tsw}")
                nc.sync.dma_start(xt, xk[ko, :, tsi])
                xts.append(xt)

        # ---- matmul: psum[:, ch, (b,t)] += W'.T @ x  (split terms) ----
        pt = psum.tile([128, 2, B * tsw], f32, tag="pt")
        for ch in range(2) if not skip_mm else ():
            n_acc = len(terms) * KO
            i_acc = 0
            for ko in range(KO):
                for (wh, xh) in terms:
                    lhsT = w_sb[:, wh, ko, ch * 128:(ch + 1) * 128]
                    first = i_acc == 0
                    last = i_acc == n_acc - 1
                    i_acc += 1
                    for nb in range(B // nbb):
                        nc.tensor.matmul(
                            pt[:, ch, nb * nbb * tsw:(nb + 1) * nbb * tsw],
                            lhsT,
                            xts[ko][:, xh, nb * nbb:(nb + 1) * nbb, :],
                            start=first,
                            stop=(last and not with_bias),
                        )
            if with_bias:
                for nb in range(B // nbb):
                    nc.tensor.matmul(
                        pt[:, ch, nb * nbb * tsw:(nb + 1) * nbb * tsw],
                        bias_sb[:, ch, :],
                        ones_sb[:, :nbb * tsw],
                        start=False,
                        stop=True,
                    )

        # ---- evacuate psum -> sbuf z-block [128, TS, 2, B] (ACT) ----
        # (scan_src == "psum": no evac; the scan's charge op reads z
        #  straight out of PSUM and writes u into zb)
        zb = zs.tile([128, tsw, 2, B], f32, tag=f"zb{tsw}")
        ptv = pt.rearrange("p c (b t) -> p t c b", t=tsw)
        if not (skip_mm or skip_evac) and scan_src != "psum":
            if evac_merge is None:
                evac_merge = "ch"
            if evac_merge == "one":
                nc.scalar.copy(
                    out=zb[:],
                    in_=pt.rearrange("p c (b t) -> p t c b", t=tsw),
                )
            elif isinstance(evac_merge, str) and evac_merge.startswith("t"):
                # per-chunk-of-steps evac: scan step ti only waits its own
                # chunk, so ACT latency pipelines off the DVE chain
                tn = int(evac_merge[1:])
                ptv = pt.rearrange("p c (b t) -> p t c b", t=tsw)
                for t0 in range(0, tsw, tn):
                    nc.scalar.copy(
                        out=zb[:, t0:t0 + tn],
                        in_=ptv[:, t0:t0 + tn],
                    )
            else:
                for ch in range(2):
                    nc.scalar.copy(
                        out=zb[:, :, ch, :],
                        in_=pt[:, ch].rearrange("p (b t) -> p t b", t=tsw),
                    )

        # ---- LIF scan: 2 fused DVE ops/step (+ count) ----
        for ti in range(tsw):
            t = t_base + ti
            u = zb[:, ti]  # holds z_t; overwritten in place with u_t
            zsrc = ptv[:, ti] if scan_src == "psum" else u
            if scan_mode == "ilv2":
                # two independent ch streams; interleave so each op's sem
                # latency hides behind the other stream's op
                if scan_ops >= 1:
                    for ch in range(2):
                        nc.vector.scalar_tensor_tensor(
                            out=u[:, ch], in0=v[:, ch], scalar=0.5,
                            in1=u[:, ch], op0=op.mult, op1=op.add,
                        )
                if scan_ops >= 3 and t >= warm and count_engine == "vector":
                    for ch in range(2):
                        nc.vector.scalar_tensor_tensor(
                            out=acc[:, ch], in0=u[:, ch], scalar=1.0,
                            in1=acc[:, ch], op0=op.is_ge, op1=op.add,
                        )
                if scan_ops >= 2:
                    for ch in range(2):
                        nc.vector.scalar_tensor_tensor(
                            out=v[:, ch], in0=u[:, ch], scalar=1.0,
                            in1=u[:, ch], op0=op.is_lt, op1=op.mult,
                        )
                continue
            do_count = scan_ops >= 3 and t >= warm and count_engine == "vector"
            if scan_ops >= 1:
                nc.vector.scalar_tensor_tensor(
                    out=u, in0=v, scalar=0.5, in1=zsrc,
                    op0=op.mult, op1=op.add,
                )
            if do_count and scan_mode != "uva":
                nc.vector.scalar_tensor_tensor(
                    out=acc, in0=u, scalar=1.0, in1=acc,
                    op0=op.is_ge, op1=op.add,
                )
            if scan_ops >= 2:
                nc.vector.scalar_tensor_tensor(
                    out=v, in0=u, scalar=1.0, in1=u,
                    op0=op.is_lt, op1=op.mult,
                )
            if do_count and scan_mode == "uva":
                # count emitted AFTER the reset: it fills the ~100ns sem
                # gap before the next step's charge op instead of sitting
                # on the u->v critical chain
                nc.vector.scalar_tensor_tensor(
                    out=acc, in0=u, scalar=1.0, in1=acc,
                    op0=op.is_ge, op1=op.add,
                )

        if count_engine == "pool_block" and t_base >= warm:
            # zb still holds all 16 u_t tiles; count spikes on GPSIMD
            mblk = zs.tile([128, tsw, 2, B], f32, tag="mblk")
            nc.gpsimd.tensor_scalar(
                out=mblk[:], in0=zb[:], scalar1=1.0, scalar2=None,
                op0=op.is_ge,
            )
            h = tsw
            while h > 1:
                h //= 2
                nc.gpsimd.tensor_tensor(
                    out=mblk[:, :h], in0=mblk[:, :h], in1=mblk[:, h:2 * h],
                    op=op.add,
                )
            nc.gpsimd.tensor_tensor(
                out=acc, in0=acc, in1=mblk[:, 0], op=op.add,
            )

        t_base += tsw

    nc.sync.dma_start(counts[:], acc)


def _split_bf16(a):
    """fp32 -> (hi, lo) bf16 pair with hi + lo ~ a (error ~2^-18 relative)."""
    import ml_dtypes
    hi = a.astype(ml_dtypes.bfloat16)
    lo = (a - hi.astype(np.float32)).astype(ml_dtypes.bfloat16)
    return hi, lo


def _prep_inputs(x, W, gamma, beta, run_mean, run_var, mm_mode=None,
                 warm=WARM, ts=TS, taper=False, fuse_dma=True):
    """Fold BN + 1/TAU into weights; build per-core time-sharded x layouts."""
    if mm_mode is None:
        mm_mode = MM_MODE
    tloc = warm + TCH
    nts = tloc // 4 if taper else tloc // ts
    tsz = 4 if taper else ts
    split = mm_mode == "bf16x3"
    import ml_dtypes

    inv = 1.0 / np.sqrt(run_var.astype(np.float64) + BN_EPS)
    a = (0.5 * gamma.astype(np.float64) * inv)
    Wp = (W.astype(np.float64) * a[:, None]).astype(np.float32)       # [COUT, CIN]
    bp = (0.5 * (beta.astype(np.float64)
                 - run_mean.astype(np.float64) * gamma.astype(np.float64) * inv)
          ).astype(np.float32)                                        # [COUT]
    wT = np.ascontiguousarray(Wp.T)                                   # [CIN, COUT]
    if split:
        wh, wl = _split_bf16(wT)
        wTs = np.ascontiguousarray(np.stack([wh, wl], axis=0))        # [2,CIN,COUT]
        xh, xl = _split_bf16(x)
        xhl = np.stack([xh, xl], axis=0)                              # [2,B,CIN,T]
    else:
        wTs = wT.reshape(1, CIN, COUT)

    in_maps = []
    for c in range(NCORES):
        t0 = c * TCH - warm
        lo = max(t0, 0)
        if split:
            xc = np.zeros((2, B, CIN, tloc), dtype=ml_dtypes.bfloat16)
            xc[:, :, :, lo - t0:] = xhl[:, :, :, lo:c * TCH + TCH]
            # [2, B, CIN, tloc] -> [KO, 128, nts, 2, B, TS]
            if taper:
                # per-window contiguous blocks concatenated on the free axis
                blocks = []
                t0 = 0
                for tsw in [4, 4] + [8] * ((tloc - 8) // 8):
                    blk = (xc[:, :, :, t0:t0 + tsw]
                           .reshape(2, B, KO, 128, tsw)
                           .transpose(3, 2, 0, 1, 4)
                           .reshape(128, -1))
                    blocks.append(blk)
                    t0 += tsw
                xkc = np.ascontiguousarray(np.concatenate(blocks, axis=1))
            elif fuse_dma:
                # [128, nts, KO, nhl, B, tsz]
                xkc = np.ascontiguousarray(
                    xc.reshape(2, B, KO, 128, nts, tsz)
                      .transpose(3, 4, 2, 0, 1, 5))
            else:
                xkc = np.ascontiguousarray(
                    xc.reshape(2, B, KO, 128, nts, tsz)
                      .transpose(2, 3, 4, 0, 1, 5))
        else:
            xc = np.zeros((B, CIN, tloc), dtype=np.float32)
            xc[:, :, lo - t0:] = x[:, :, lo:c * TCH + TCH]
            xkc = np.ascontiguousarray(
                xc.reshape(B, KO, 128, nts, tsz).transpose(1, 2, 3, 0, 4)
            )[:, :, :, None]
        m = {"xk": xkc, "wT": wTs}
        if np.any(bp != 0):
            m["bvec"] = np.ascontiguousarray(bp.reshape(1, 2, 128))
        in_maps.append(m)
    return in_maps, bool(np.any(bp != 0))


def _postprocess(results):
    total = np.zeros((128, 2, B), dtype=np.float64)
    for r in results:
        total += r["counts"].astype(np.float64)
    # counts[ci, ch, b] -> out[b, ch*128+ci]
    out = total.transpose(2, 1, 0).reshape(B, COUT) / float(T)
    return out.astype(np.float32)


def kernel(x, W, gamma, beta, run_mean, run_var, _trace=False):
    from concourse.bass_utils import run_bass_kernel_spmd

    x = np.asarray(x, dtype=np.float32)
    W = np.asarray(W, dtype=np.float32)
    gamma = np.asarray(gamma, dtype=np.float32)
    beta = np.asarray(beta, dtype=np.float32)
    run_mean = np.asarray(run_mean, dtype=np.float32)
    run_var = np.asarray(run_var, dtype=np.float32)

    in_maps, with_bias = _prep_inputs(x, W, gamma, beta, run_mean, run_var)
    key = ("nc", with_bias, MM_MODE, COUNT_ENGINE, SCAN_MODE, EVAC_MODE,
           SCAN_SRC)
    if key not in _CACHE:
        _CACHE[key] = _build_nc(with_bias, evac_merge=EVAC_MODE,
                                scan_src=SCAN_SRC)
    nc = _CACHE[key]

    res = run_bass_kernel_spmd(
        nc, in_maps, core_ids=list(range(NCORES)), trace=_trace
    )
    out = _postprocess(res.results)
    if _trace:
        return out, res
    return out


if __name__ == "__main__":
    rng = np.random.default_rng(0)
    x = rng.standard_normal((B, CIN, T), dtype=np.float32)
    W = (rng.standard_normal((COUT, CIN), dtype=np.float32) / np.sqrt(CIN)).astype(np.float32)
    out = kernel(x, W, np.ones(COUT, np.float32), np.zeros(COUT, np.float32),
                 np.zeros(COUT, np.float32), np.ones(COUT, np.float32))
    print(out.shape, out.dtype, out[:2, :4])

